# revision 37
# baseline (speedup 1.0000x reference)
"""Trainium2 Bass kernel for nn_AltDiff (FC -> 50-iter ADMM QP solve -> FC -> log_softmax).

Strategy
--------
Pure data parallelism over the batch (8192 rows -> 1024 per NeuronCore on 8
cores); all solver matrices are tiny and replicated. The per-sample math is
algebraically compressed on the host (float64):

  ADMM state (s, lam, nu) is replaced by (q, r, lam) with
    s = relu(q) = r,   nu = relu(-q) = r - q,
  so each of the 49 iterations is ONE affine map
    [q'; lam'] = W_X @ [q; lam; e_top] + W_Y @ [r; e_bot]
  evaluated as two accumulating K=128 matmuls per 512-column half-batch
  (the per-sample constant e = D_p @ p + dconst rides along in spare
  K-partitions with identity weight blocks), followed by
    ACT:  [q'; lam'] copy PSUM->SBUF,   DVE:  r' = max(q', 0).
  Iteration 1 (state = 0) collapses to [q1; lam1] = e and seeds the static
  e-rows. After iteration 49, z is reconstructed with three matmuls; the
  final FC + log_softmax run batched with classes in the free axis
  (no max-subtraction: |logits| < 20, exp is fp32-safe).

Matmul operands are float16 (full-rate PE, preloadable weights); PSUM
accumulation and all elementwise arithmetic stay fp32. Validated vs the
float64 reference: rel err ~9.4e-4; measured 106.7us on 8 cores (from a
221us first working version: fp32r->fp16, batched epilogue, warm-up
matmuls, DMA priority ordering, and writeback restructured to
ACT-full-copy + DVE 4x-mode relu).
"""

import numpy as np

B, NF, NH, NC = 8192, 512, 128, 10
NEQ, NINEQ = 32, 64
N_ITERS = 50
NCORES = 8
BL = B // NCORES          # batch rows per core
HALF = 512                # matmul free-dim chunk (one PSUM bank of fp32)
NCHUNK = BL // HALF       # 2
NLG = BL // 128           # 8 log_softmax row-chunks


def _host_precompute(fc1_w, fc1_b, fc2_w, fc2_b, G, h, A, b):
    """Build all replicated device constants in float64, return device dtypes."""
    f8 = np.float64
    G, h, A, b = (np.asarray(t, f8) for t in (G, h, A, b))
    fc1_w = np.asarray(fc1_w, f8)
    fc2_w = np.asarray(fc2_w, f8)
    K = 0.1 * np.eye(NH) + A.T @ A + G.T @ G
    Kinv = np.linalg.inv(K)
    M_A = Kinv @ A.T            # [128, 32]
    M_G = Kinv @ G.T            # [128, 64]
    S_GG = G @ M_G
    S_GA = G @ M_A
    S_AG = A @ M_G
    S_AA = A @ M_A
    P_G = G @ Kinv              # [64, 128]
    P_A = A @ Kinv              # [32, 128]
    c0 = Kinv @ (A.T @ b)
    g0 = G @ (c0 + M_G @ h)
    a0 = A @ (c0 + M_G @ h)
    I64, I32 = np.eye(64), np.eye(32)

    # Iteration map [q'; lam'] = W_X @ [q; lam; e_top] + W_Y @ [r; e_bot]
    W_X = np.zeros((128, 128))
    W_X[0:96, 0:64] = np.vstack([I64 - S_GG, S_AG])        # coeff of q
    W_X[0:96, 64:96] = np.vstack([S_GA, I32 - S_AA])       # coeff of lam
    W_X[0:32, 96:128] = I32                                # + e[0:32]
    W_Y = np.zeros((128, 128))
    W_Y[0:96, 0:64] = np.vstack([2 * S_GG - I64, -2 * S_AG])  # coeff of r
    W_Y[32:96, 64:128] = I64                               # + e[32:96]

    D_p = np.zeros((128, 128))
    D_p[0:96] = np.vstack([P_G, -P_A])
    dconst = np.concatenate([h - g0, a0 - b])              # [96]

    # Iteration 49 is folded into the z-reconstruction:
    #   z = zb - Kinv@p + M_G@q49 - 2 M_G@r49 - M_A@lam49
    # with [q49; lam49] = W_X@state48 + W_Y@state48 composed on the host, so
    # only r49 = relu(q49) needs an on-device writeback.
    W_ZX = M_G @ W_X[0:64, :] - M_A @ W_X[64:96, :]      # [128, 128]
    W_ZY = M_G @ W_Y[0:64, :] - M_A @ W_Y[64:96, :]      # [128, 128]
    W_ZR = -2 * M_G                                      # [128, 64]
    zb = c0 + M_G @ h

    # fc1 lhsT chunks: [128 k, 4*128 m] with chunk c in cols c*128:(c+1)*128
    w1T = np.concatenate(
        [fc1_w.T[c * 128:(c + 1) * 128, :] for c in range(4)], axis=1
    )

    f4, f2 = np.float32, np.float16
    return {
        "w1T": np.ascontiguousarray(w1T, f2),
        "b1": np.asarray(fc1_b, f4).reshape(NH, 1),
        "lhsX": np.ascontiguousarray(W_X.T, f2),
        "lhsY": np.ascontiguousarray(W_Y.T, f2),
        "lhsE": np.ascontiguousarray(D_p.T, f2),
        "dconst": np.asarray(dconst, f4).reshape(96, 1),
        "lhsZp": np.ascontiguousarray(-Kinv, f2),          # (-Kinv).T == -Kinv
        "lhsZx": np.ascontiguousarray(W_ZX.T, f2),
        "lhsZy": np.ascontiguousarray(W_ZY.T, f2),
        "lhsZr": np.ascontiguousarray(W_ZR.T, f2),         # [64, 128]
        "zb": np.asarray(zb, f4).reshape(NH, 1),
        "w2T": np.ascontiguousarray(np.asarray(fc2_w).T, f2),  # [128, 10]
        "b2bc": np.ascontiguousarray(
            np.broadcast_to(np.asarray(fc2_b, f4), (128, NC))
        ),
    }


# (name, shape, is_matmul_operand) — matmul operands are float16 typed
_CONST_NAMES = [
    ("w1T", [128, 512], True),
    ("b1", [128, 1], False),
    ("lhsX", [128, 128], True),
    ("lhsY", [128, 128], True),
    ("lhsE", [128, 128], True),
    ("dconst", [96, 1], False),
    ("lhsZp", [128, 128], True),
    ("lhsZx", [128, 128], True),
    ("lhsZy", [128, 128], True),
    ("lhsZr", [64, 128], True),
    ("zb", [128, 1], False),
    ("w2T", [128, NC], True),
    ("b2bc", [128, NC], False),
]

_BUILT = {}


def build_nc():
    if "nc" in _BUILT:
        return _BUILT["nc"]
    import concourse.bass as bass
    import concourse.mybir as mybir
    from concourse import bacc, tile

    f32 = mybir.dt.float32
    f16 = mybir.dt.float16
    AF = mybir.ActivationFunctionType
    Alu = mybir.AluOpType
    X = mybir.AxisListType.X

    nc = bacc.Bacc("TRN2", debug=False, target_bir_lowering=False)

    xT = nc.declare_dram_parameter("xT", [128, 4 * BL], f16, isOutput=False)
    cst = {
        name: nc.declare_dram_parameter(name, shape, f16 if is_mm else f32, isOutput=False)
        for name, shape, is_mm in _CONST_NAMES
    }
    # Output stays in on-chip layout [128 rows, chunk, class]; the host
    # unshuffles. A [BL, NC] layout would need 1024 strided 40-byte DMA
    # descriptors; this is one contiguous transfer.
    out_d = nc.declare_dram_parameter("out", [128, NLG * NC], f32, isOutput=True)

    with tile.TileContext(nc) as tc:
        with (
            tc.tile_pool(name="consts", bufs=1) as consts,
            tc.tile_pool(name="data", bufs=1) as data,
            tc.tile_pool(name="ps", bufs=6, space="PSUM") as pspool,
            tc.tile_pool(name="pslg", bufs=1, space="PSUM") as pslgpool,
            tc.tile_pool(name="work", bufs=1) as work,
        ):
            # PE warm-up: ~10 matmuls on a zeroed tile so the HAM clock-gate
            # opens while the input DMA streams in.
            warm = data.tile([128, HALF], f16, tag="warm")
            nc.vector.memset(warm[:, :], 0.0)
            warm_ps = pspool.tile([128, HALF], f32, tag="ps")
            for _ in range(6):
                nc.tensor.matmul(
                    warm_ps[:, :], lhsT=warm[:, 0:128], rhs=warm[:, :],
                    start=True, stop=True,
                )

            # DMA priority order: x chunk 0 + fc1 weights first so the first
            # fc1 matmul can start while the rest of x streams in.
            xT_sb = data.tile([128, 4 * BL], f16, tag="xT")
            csb = {}
            for name, shape, is_mm in _CONST_NAMES:
                t = consts.tile(shape, f16 if is_mm else f32, tag=name)
                csb[name] = t
            nc.sync.dma_start(out=csb["w1T"][:], in_=cst["w1T"][:])
            nc.sync.dma_start(out=csb["b1"][:], in_=cst["b1"][:])
            # x arrives grouped by half-batch (host layout [h, c, 512]): two
            # fully contiguous 512KB DMAs; fc1's h0 matmuls start after the
            # first one while h1 streams behind it.
            for hx in range(NCHUNK):
                s0 = hx * (4 * HALF)
                nc.sync.dma_start(
                    out=xT_sb[:, s0:s0 + 4 * HALF], in_=xT[:, s0:s0 + 4 * HALF]
                )
            for name, shape, is_mm in _CONST_NAMES:
                if name in ("w1T", "b1"):
                    continue
                nc.sync.dma_start(out=csb[name][:], in_=cst[name][:])

            # keep the warm-up matmuls alive (fake consumer, overwritten later)
            warm_sink = data.tile([1, 1], f32, tag="wsink")
            nc.scalar.copy(out=warm_sink[:, :], in_=warm_ps[0:1, 0:1])

            # ---- p = relu(W1 @ x^T + b1), feature-major [128, BL] ----
            pT_sb = data.tile([128, BL], f16, tag="pT")
            for hf in range(NCHUNK):
                ps = pspool.tile([128, HALF], f32, tag="ps")
                for c in range(4):
                    s0 = hf * (4 * HALF) + c * HALF
                    nc.tensor.matmul(
                        ps[:, :],
                        lhsT=csb["w1T"][:, c * 128:(c + 1) * 128],
                        rhs=xT_sb[:, s0:s0 + HALF],
                        start=(c == 0),
                        stop=(c == 3),
                    )
                nc.scalar.activation(
                    out=pT_sb[:, hf * HALF:(hf + 1) * HALF],
                    in_=ps[:, :],
                    func=AF.Relu,
                    bias=csb["b1"][:, :],
                    scale=1.0,
                )

            # ---- iteration 1 (state=0): [q1; lam1] = e = D_p @ p + dconst ----
            # Also seeds the static rows: X e_top = e[0:32], Y e_bot = e[32:96].
            # X rows: q 0:64 | lam 64:96 | e_top 96:128
            # Y rows: r 0:64 | e_bot 64:128
            X_sb = data.tile([128, BL], f16, tag="X")
            Y_sb = data.tile([128, BL], f16, tag="Y")
            for hf in range(NCHUNK):
                sl = slice(hf * HALF, (hf + 1) * HALF)
                ps = pspool.tile([128, HALF], f32, tag="ps")
                nc.tensor.matmul(
                    ps[:, :], lhsT=csb["lhsE"][:, :], rhs=pT_sb[:, sl],
                    start=True, stop=True,
                )
                nc.scalar.activation(
                    out=X_sb[0:96, sl], in_=ps[0:96, :],
                    func=AF.Identity, bias=csb["dconst"][0:96, :], scale=1.0,
                )
                nc.scalar.activation(
                    out=X_sb[96:128, sl], in_=ps[0:32, :],
                    func=AF.Identity, bias=csb["dconst"][0:32, :], scale=1.0,
                )
                nc.scalar.activation(
                    out=Y_sb[64:96, sl], in_=ps[32:64, :],
                    func=AF.Identity, bias=csb["dconst"][32:64, :], scale=1.0,
                )
                nc.scalar.activation(
                    out=Y_sb[96:128, sl], in_=ps[64:96, :],
                    func=AF.Identity, bias=csb["dconst"][64:96, :], scale=1.0,
                )
                nc.vector.tensor_scalar(
                    out=Y_sb[0:64, sl], in0=ps[0:64, :],
                    scalar1=csb["dconst"][0:64, :], scalar2=0.0,
                    op0=Alu.add, op1=Alu.max,
                )

            # ---- iterations 2..48 ----
            # Writeback is latency-critical (next matmul waits on X): split the
            # [96, 512] PSUM->SBUF copy between ACT (left half) and DVE (right
            # half), then GpSimd derives r' = relu(q') SBUF->SBUF off the
            # critical path.
            for _ in range(N_ITERS - 3):
                for hf in range(NCHUNK):
                    SPL = 512  # ACT/DVE copy split; relu waits on the longer one
                    sl = slice(hf * HALF, (hf + 1) * HALF)
                    sl_l = slice(hf * HALF, hf * HALF + SPL)
                    sl_r = slice(hf * HALF + SPL, (hf + 1) * HALF)
                    ps = pspool.tile([128, HALF], f32, tag="ps")
                    nc.tensor.matmul(
                        ps[:, :], lhsT=csb["lhsX"][:, :], rhs=X_sb[:, sl],
                        start=True, stop=False,
                    )
                    nc.tensor.matmul(
                        ps[:, :], lhsT=csb["lhsY"][:, :], rhs=Y_sb[:, sl],
                        start=False, stop=True,
                    )
                    nc.scalar.copy(out=X_sb[0:96, sl_l], in_=ps[0:96, 0:SPL])
                    if SPL < HALF:
                        nc.vector.tensor_copy(
                            out=X_sb[0:96, sl_r], in_=ps[0:96, SPL:HALF]
                        )
                    nc.vector.tensor_scalar_max(
                        out=Y_sb[0:64, sl], in0=X_sb[0:64, sl], scalar1=0.0
                    )

            # ---- iteration 49: only r49 = relu(q49) is materialized ----
            r49_sb = data.tile([64, BL], f16, tag="r49")
            for hf in range(NCHUNK):
                sl = slice(hf * HALF, (hf + 1) * HALF)
                ps = pspool.tile([128, HALF], f32, tag="ps")
                nc.tensor.matmul(
                    ps[:, :], lhsT=csb["lhsX"][:, :], rhs=X_sb[:, sl],
                    start=True, stop=False,
                )
                nc.tensor.matmul(
                    ps[:, :], lhsT=csb["lhsY"][:, :], rhs=Y_sb[:, sl],
                    start=False, stop=True,
                )
                nc.vector.tensor_scalar_max(
                    out=r49_sb[0:64, sl], in0=ps[0:64, :], scalar1=0.0
                )

            # ---- z = zb - Kinv@p + W_ZX@state48_X + W_ZY@state48_Y - 2 M_G@r49 ----
            zT_sb = data.tile([128, BL], f16, tag="zT")
            for hf in range(NCHUNK):
                sl = slice(hf * HALF, (hf + 1) * HALF)
                ps = pspool.tile([128, HALF], f32, tag="ps")
                nc.tensor.matmul(
                    ps[:, :], lhsT=csb["lhsZp"][:, :], rhs=pT_sb[:, sl],
                    start=True, stop=False,
                )
                nc.tensor.matmul(
                    ps[:, :], lhsT=csb["lhsZx"][:, :], rhs=X_sb[:, sl],
                    start=False, stop=False,
                )
                nc.tensor.matmul(
                    ps[:, :], lhsT=csb["lhsZy"][:, :], rhs=Y_sb[:, sl],
                    start=False, stop=False,
                )
                nc.tensor.matmul(
                    ps[:, :], lhsT=csb["lhsZr"][:, :], rhs=r49_sb[0:64, sl],
                    start=False, stop=True,
                )
                nc.scalar.activation(
                    out=zT_sb[:, sl], in_=ps[:, :],
                    func=AF.Identity, bias=csb["zb"][:, :], scale=1.0,
                )

            # ---- logits for all 1024 rows into one PSUM bank [128, 8*10] ----
            pslg = pslgpool.tile([128, NLG * NC], f32, tag="pslg")
            for ch in range(NLG):
                nc.tensor.matmul(
                    pslg[:, ch * NC:(ch + 1) * NC],
                    lhsT=zT_sb[:, ch * 128:(ch + 1) * 128],
                    rhs=csb["w2T"][:, :],
                    start=True, stop=True,
                )
            lg = work.tile([128, NLG, NC], f32, tag="lg")
            # logits + fc2 bias (b2bc broadcast across the chunk dim)
            b2b = csb["b2bc"][:, :]
            b2_bcast = bass.AP(
                tensor=b2b.tensor, offset=b2b.offset,
                ap=[b2b.ap[0], [0, NLG], b2b.ap[1]],
            )
            nc.vector.tensor_tensor(
                out=lg[:, :, :],
                in0=pslg[:, :].rearrange("p (c n) -> p c n", c=NLG),
                in1=b2_bcast, op=Alu.add,
            )
            # log_softmax without max-subtraction (|logits| < 20)
            ex = work.tile([128, NLG, NC], f32, tag="ex")
            nc.scalar.activation(out=ex[:, :, :], in_=lg[:, :, :], func=AF.Exp)
            sm = work.tile([128, NLG], f32, tag="sm")
            nc.vector.tensor_reduce(
                out=sm[:, :], in_=ex[:, :, :], axis=X, op=Alu.add
            )
            lnv = work.tile([128, NLG], f32, tag="lnv")
            nc.scalar.activation(out=lnv[:, :], in_=sm[:, :], func=AF.Ln)
            ot = work.tile([128, NLG, NC], f32, tag="ot")
            lnv_ap = lnv[:, :]
            lnv_bcast = bass.AP(
                tensor=lnv_ap.tensor, offset=lnv_ap.offset,
                ap=[lnv_ap.ap[0], lnv_ap.ap[1], [0, NC]],
            )
            nc.vector.tensor_tensor(
                out=ot[:, :, :], in0=lg[:, :, :], in1=lnv_bcast, op=Alu.subtract
            )
            nc.sync.dma_start(
                out=out_d[:, :],
                in_=ot[:, :, :].rearrange("p c n -> p (c n)"),
            )

    nc.compile()
    _BUILT["nc"] = nc
    return nc


def make_in_maps(x, consts):
    """Shard x over cores; constants replicated."""
    x = np.asarray(x, np.float32)
    in_maps = []
    for c in range(NCORES):
        shard = x[c * BL:(c + 1) * BL]                 # [BL, 512]
        xs = shard.T                                   # [512, BL]
        # layout [128, (h, chunk, HALF)]: each half-batch contiguous
        xTc = np.concatenate(
            [xs[k * 128:(k + 1) * 128, h * HALF:(h + 1) * HALF]
             for h in range(NCHUNK) for k in range(4)],
            axis=1,
        )
        m = {"xT": np.ascontiguousarray(xTc, np.float16)}
        m.update(consts)
        in_maps.append(m)
    return in_maps


def _ensure_axon_hooks():
    """`run_bass_kernel_spmd(trace=True)` under axon imports
    antenv.axon_hooks, which this image lacks. Register a working hook if
    the boot helper is available, else a stub so tracing degrades instead
    of crashing."""
    import sys
    import types

    try:
        import antenv.axon_hooks  # noqa: F401
        return
    except ImportError:
        pass

    hook = None
    try:
        from trn_agent_boot.trn_boot import _ntff_profile_via_ctypes
        import os
        so = "/opt/axon/libaxon_pjrt.so"
        if os.path.exists(so):
            hook = _ntff_profile_via_ctypes(so)
    except Exception:
        hook = None

    m = types.ModuleType("antenv.axon_hooks")
    m.get_axon_ntff_profile_hook = lambda: hook
    m.set_axon_ntff_profile_hook = lambda h: None
    sys.modules["antenv.axon_hooks"] = m


def gather_out(results):
    """Device output is [128, chunk, class]; restore [B, NC] row order."""
    shards = []
    for c in range(NCORES):
        o = np.asarray(results[c]["out"]).reshape(128, NLG, NC)
        shards.append(np.transpose(o, (1, 0, 2)).reshape(BL, NC))
    return np.concatenate(shards, axis=0)


def kernel(x, fc1_w, fc1_b, fc2_w, fc2_b, G, h, A, b):
    from concourse.bass_utils import run_bass_kernel_spmd

    _ensure_axon_hooks()
    consts = _host_precompute(fc1_w, fc1_b, fc2_w, fc2_b, G, h, A, b)
    nc = build_nc()
    in_maps = make_in_maps(x, consts)
    res = run_bass_kernel_spmd(nc, in_maps, core_ids=list(range(NCORES)))
    return gather_out(res.results).astype(np.float32)


# revision 38
# speedup vs baseline: 1.1796x; 1.1796x over previous
"""Trainium2 Bass kernel for nn_AltDiff (FC -> 50-iter ADMM QP solve -> FC -> log_softmax).

Strategy
--------
Pure data parallelism over the batch (8192 rows -> 1024 per NeuronCore on 8
cores); all solver matrices are tiny and replicated. The per-sample math is
algebraically compressed on the host (float64):

  ADMM state (s, lam, nu) is replaced by (q, r, lam) with
    s = relu(q) = r,   nu = relu(-q) = r - q,
  so each of the 49 iterations is ONE affine map
    [q'; lam'] = W_X @ [q; lam; e_top] + W_Y @ [r; e_bot]
  evaluated as two accumulating K=128 matmuls per 512-column half-batch
  (the per-sample constant e = D_p @ p + dconst rides along in spare
  K-partitions with identity weight blocks), followed by
    ACT:  [q'; lam'] copy PSUM->SBUF,   DVE:  r' = max(q', 0).
  Iteration 1 (state = 0) collapses to [q1; lam1] = e and seeds the static
  e-rows. After iteration 49, z is reconstructed with three matmuls; the
  final FC + log_softmax run batched with classes in the free axis
  (no max-subtraction: |logits| < 20, exp is fp32-safe).

Matmul operands are float16 (full-rate PE, preloadable weights); PSUM
accumulation and all elementwise arithmetic stay fp32. Validated vs the
float64 reference: rel err ~9.4e-4; measured 106.7us on 8 cores (from a
221us first working version: fp32r->fp16, batched epilogue, warm-up
matmuls, DMA priority ordering, and writeback restructured to
ACT-full-copy + DVE 4x-mode relu).
"""

import numpy as np

B, NF, NH, NC = 8192, 512, 128, 10
NEQ, NINEQ = 32, 64
N_ITERS = 50
NCORES = 8
BL = B // NCORES          # batch rows per core
HALF = 512                # matmul free-dim chunk (one PSUM bank of fp32)
NCHUNK = BL // HALF       # 2
NLG = BL // 128           # 8 log_softmax row-chunks


def _host_precompute(fc1_w, fc1_b, fc2_w, fc2_b, G, h, A, b):
    """Build all replicated device constants in float64, return device dtypes."""
    f8 = np.float64
    G, h, A, b = (np.asarray(t, f8) for t in (G, h, A, b))
    fc1_w = np.asarray(fc1_w, f8)
    fc2_w = np.asarray(fc2_w, f8)
    K = 0.1 * np.eye(NH) + A.T @ A + G.T @ G
    Kinv = np.linalg.inv(K)
    M_A = Kinv @ A.T            # [128, 32]
    M_G = Kinv @ G.T            # [128, 64]
    S_GG = G @ M_G
    S_GA = G @ M_A
    S_AG = A @ M_G
    S_AA = A @ M_A
    P_G = G @ Kinv              # [64, 128]
    P_A = A @ Kinv              # [32, 128]
    c0 = Kinv @ (A.T @ b)
    g0 = G @ (c0 + M_G @ h)
    a0 = A @ (c0 + M_G @ h)
    I64, I32 = np.eye(64), np.eye(32)

    # Iteration map [q'; lam'] = W_X @ [q; lam; e_top] + W_Y @ [r; e_bot]
    W_X = np.zeros((128, 128))
    W_X[0:96, 0:64] = np.vstack([I64 - S_GG, S_AG])        # coeff of q
    W_X[0:96, 64:96] = np.vstack([S_GA, I32 - S_AA])       # coeff of lam
    W_X[0:32, 96:128] = I32                                # + e[0:32]
    W_Y = np.zeros((128, 128))
    W_Y[0:96, 0:64] = np.vstack([2 * S_GG - I64, -2 * S_AG])  # coeff of r
    W_Y[32:96, 64:128] = I64                               # + e[32:96]

    D_p = np.zeros((128, 128))
    D_p[0:96] = np.vstack([P_G, -P_A])
    dconst = np.concatenate([h - g0, a0 - b])              # [96]

    # Iteration 49 is folded into the z-reconstruction:
    #   z = zb - Kinv@p + M_G@q49 - 2 M_G@r49 - M_A@lam49
    # with [q49; lam49] = W_X@state48 + W_Y@state48 composed on the host, so
    # only r49 = relu(q49) needs an on-device writeback.
    W_ZX = M_G @ W_X[0:64, :] - M_A @ W_X[64:96, :]      # [128, 128]
    W_ZY = M_G @ W_Y[0:64, :] - M_A @ W_Y[64:96, :]      # [128, 128]
    W_ZR = -2 * M_G                                      # [128, 64]
    zb = c0 + M_G @ h

    # fc1 lhsT chunks: [128 k, 4*128 m] with chunk c in cols c*128:(c+1)*128
    w1T = np.concatenate(
        [fc1_w.T[c * 128:(c + 1) * 128, :] for c in range(4)], axis=1
    )

    f4, f2 = np.float32, np.float16
    return {
        "w1T": np.ascontiguousarray(w1T, f2),
        "b1": np.asarray(fc1_b, f4).reshape(NH, 1),
        "lhsX": np.ascontiguousarray(W_X.T, f2),
        "lhsY": np.ascontiguousarray(W_Y.T, f2),
        "lhsE": np.ascontiguousarray(D_p.T, f2),
        "dconst": np.asarray(dconst, f4).reshape(96, 1),
        "lhsZp": np.ascontiguousarray(-Kinv, f2),          # (-Kinv).T == -Kinv
        "lhsZx": np.ascontiguousarray(W_ZX.T, f2),
        "lhsZy": np.ascontiguousarray(W_ZY.T, f2),
        "lhsZr": np.ascontiguousarray(W_ZR.T, f2),         # [64, 128]
        "zb": np.asarray(zb, f4).reshape(NH, 1),
        "w2T": np.ascontiguousarray(np.asarray(fc2_w).T, f2),  # [128, 10]
        "b2bc": np.ascontiguousarray(
            np.broadcast_to(np.asarray(fc2_b, f4), (128, NC))
        ),
    }


# (name, shape, is_matmul_operand) — matmul operands are float16 typed
_CONST_NAMES = [
    ("w1T", [128, 512], True),
    ("b1", [128, 1], False),
    ("lhsX", [128, 128], True),
    ("lhsY", [128, 128], True),
    ("lhsE", [128, 128], True),
    ("dconst", [96, 1], False),
    ("lhsZp", [128, 128], True),
    ("lhsZx", [128, 128], True),
    ("lhsZy", [128, 128], True),
    ("lhsZr", [64, 128], True),
    ("zb", [128, 1], False),
    ("w2T", [128, NC], True),
    ("b2bc", [128, NC], False),
]

_BUILT = {}


def build_nc():
    if "nc" in _BUILT:
        return _BUILT["nc"]
    import concourse.bass as bass
    import concourse.mybir as mybir
    from concourse import bacc, tile

    f32 = mybir.dt.float32
    f16 = mybir.dt.float16
    AF = mybir.ActivationFunctionType
    Alu = mybir.AluOpType
    X = mybir.AxisListType.X

    nc = bacc.Bacc("TRN2", debug=False, target_bir_lowering=False)

    xT = nc.declare_dram_parameter("xT", [128, 4 * BL], f16, isOutput=False)
    cst = {
        name: nc.declare_dram_parameter(name, shape, f16 if is_mm else f32, isOutput=False)
        for name, shape, is_mm in _CONST_NAMES
    }
    # Output stays in on-chip layout [128 rows, chunk, class]; the host
    # unshuffles. A [BL, NC] layout would need 1024 strided 40-byte DMA
    # descriptors; this is one contiguous transfer.
    out_d = nc.declare_dram_parameter("out", [128, NLG * NC], f32, isOutput=True)

    with tile.TileContext(nc) as tc:
        with (
            tc.tile_pool(name="consts", bufs=1) as consts,
            tc.tile_pool(name="data", bufs=1) as data,
            tc.tile_pool(name="ps", bufs=6, space="PSUM") as pspool,
            tc.tile_pool(name="pslg", bufs=1, space="PSUM") as pslgpool,
            tc.tile_pool(name="work", bufs=1) as work,
        ):
            # PE warm-up: ~10 matmuls on a zeroed tile so the HAM clock-gate
            # opens while the input DMA streams in.
            warm = data.tile([128, HALF], f16, tag="warm")
            nc.vector.memset(warm[:, :], 0.0)
            warm_ps = pspool.tile([128, HALF], f32, tag="ps")
            for _ in range(6):
                nc.tensor.matmul(
                    warm_ps[:, :], lhsT=warm[:, 0:128], rhs=warm[:, :],
                    start=True, stop=True,
                )

            # DMA priority order: x chunk 0 + fc1 weights first so the first
            # fc1 matmul can start while the rest of x streams in.
            xT_sb = data.tile([128, 4 * BL], f16, tag="xT")
            csb = {}
            for name, shape, is_mm in _CONST_NAMES:
                t = consts.tile(shape, f16 if is_mm else f32, tag=name)
                csb[name] = t
            nc.sync.dma_start(out=csb["w1T"][:], in_=cst["w1T"][:])
            nc.sync.dma_start(out=csb["b1"][:], in_=cst["b1"][:])
            # x arrives grouped by half-batch (host layout [h, c, 512]); split
            # each half's block into 4 DMAs for queue-level overlap.
            for hx in range(NCHUNK):
                for c in range(4):
                    s0 = hx * (4 * HALF) + c * HALF
                    nc.sync.dma_start(
                        out=xT_sb[:, s0:s0 + HALF], in_=xT[:, s0:s0 + HALF]
                    )
            for name, shape, is_mm in _CONST_NAMES:
                if name in ("w1T", "b1"):
                    continue
                nc.sync.dma_start(out=csb[name][:], in_=cst[name][:])

            # keep the warm-up matmuls alive (fake consumer, overwritten later)
            warm_sink = data.tile([1, 1], f32, tag="wsink")
            nc.scalar.copy(out=warm_sink[:, :], in_=warm_ps[0:1, 0:1])

            # ---- p = relu(W1 @ x^T + b1), feature-major [128, BL] ----
            pT_sb = data.tile([128, BL], f16, tag="pT")
            for hf in range(NCHUNK):
                ps = pspool.tile([128, HALF], f32, tag="ps")
                for c in range(4):
                    s0 = hf * (4 * HALF) + c * HALF
                    nc.tensor.matmul(
                        ps[:, :],
                        lhsT=csb["w1T"][:, c * 128:(c + 1) * 128],
                        rhs=xT_sb[:, s0:s0 + HALF],
                        start=(c == 0),
                        stop=(c == 3),
                    )
                nc.scalar.activation(
                    out=pT_sb[:, hf * HALF:(hf + 1) * HALF],
                    in_=ps[:, :],
                    func=AF.Relu,
                    bias=csb["b1"][:, :],
                    scale=1.0,
                )

            # ---- iteration 1 (state=0): [q1; lam1] = e = D_p @ p + dconst ----
            # Also seeds the static rows: X e_top = e[0:32], Y e_bot = e[32:96].
            # X rows: q 0:64 | lam 64:96 | e_top 96:128
            # Y rows: r 0:64 | e_bot 64:128
            X_sb = data.tile([128, BL], f16, tag="X")
            Y_sb = data.tile([128, BL], f16, tag="Y")
            for hf in range(NCHUNK):
                sl = slice(hf * HALF, (hf + 1) * HALF)
                ps = pspool.tile([128, HALF], f32, tag="ps")
                nc.tensor.matmul(
                    ps[:, :], lhsT=csb["lhsE"][:, :], rhs=pT_sb[:, sl],
                    start=True, stop=True,
                )
                nc.scalar.activation(
                    out=X_sb[0:96, sl], in_=ps[0:96, :],
                    func=AF.Identity, bias=csb["dconst"][0:96, :], scale=1.0,
                )
                nc.scalar.activation(
                    out=X_sb[96:128, sl], in_=ps[0:32, :],
                    func=AF.Identity, bias=csb["dconst"][0:32, :], scale=1.0,
                )
                nc.scalar.activation(
                    out=Y_sb[64:96, sl], in_=ps[32:64, :],
                    func=AF.Identity, bias=csb["dconst"][32:64, :], scale=1.0,
                )
                nc.scalar.activation(
                    out=Y_sb[96:128, sl], in_=ps[64:96, :],
                    func=AF.Identity, bias=csb["dconst"][64:96, :], scale=1.0,
                )
                nc.vector.tensor_scalar(
                    out=Y_sb[0:64, sl], in0=ps[0:64, :],
                    scalar1=csb["dconst"][0:64, :], scalar2=0.0,
                    op0=Alu.add, op1=Alu.max,
                )

            # ---- iterations 2..48 ----
            # Writeback is latency-critical (next matmul waits on X): split the
            # [96, 512] PSUM->SBUF copy between ACT (left half) and DVE (right
            # half), then GpSimd derives r' = relu(q') SBUF->SBUF off the
            # critical path.
            for _ in range(N_ITERS - 3):
                for hf in range(NCHUNK):
                    SPL = 512  # ACT/DVE copy split; relu waits on the longer one
                    sl = slice(hf * HALF, (hf + 1) * HALF)
                    sl_l = slice(hf * HALF, hf * HALF + SPL)
                    sl_r = slice(hf * HALF + SPL, (hf + 1) * HALF)
                    ps = pspool.tile([128, HALF], f32, tag="ps")
                    nc.tensor.matmul(
                        ps[:, :], lhsT=csb["lhsX"][:, :], rhs=X_sb[:, sl],
                        start=True, stop=False,
                    )
                    nc.tensor.matmul(
                        ps[:, :], lhsT=csb["lhsY"][:, :], rhs=Y_sb[:, sl],
                        start=False, stop=True,
                    )
                    nc.scalar.copy(out=X_sb[0:96, sl_l], in_=ps[0:96, 0:SPL])
                    if SPL < HALF:
                        nc.vector.tensor_copy(
                            out=X_sb[0:96, sl_r], in_=ps[0:96, SPL:HALF]
                        )
                    nc.vector.tensor_scalar_max(
                        out=Y_sb[0:64, sl], in0=X_sb[0:64, sl], scalar1=0.0
                    )

            # ---- iteration 49: only r49 = relu(q49) is materialized ----
            r49_sb = data.tile([64, BL], f16, tag="r49")
            for hf in range(NCHUNK):
                sl = slice(hf * HALF, (hf + 1) * HALF)
                ps = pspool.tile([128, HALF], f32, tag="ps")
                nc.tensor.matmul(
                    ps[:, :], lhsT=csb["lhsX"][:, :], rhs=X_sb[:, sl],
                    start=True, stop=False,
                )
                nc.tensor.matmul(
                    ps[:, :], lhsT=csb["lhsY"][:, :], rhs=Y_sb[:, sl],
                    start=False, stop=True,
                )
                nc.vector.tensor_scalar_max(
                    out=r49_sb[0:64, sl], in0=ps[0:64, :], scalar1=0.0
                )

            # ---- z = zb - Kinv@p + W_ZX@state48_X + W_ZY@state48_Y - 2 M_G@r49 ----
            zT_sb = data.tile([128, BL], f16, tag="zT")
            for hf in range(NCHUNK):
                sl = slice(hf * HALF, (hf + 1) * HALF)
                ps = pspool.tile([128, HALF], f32, tag="ps")
                nc.tensor.matmul(
                    ps[:, :], lhsT=csb["lhsZp"][:, :], rhs=pT_sb[:, sl],
                    start=True, stop=False,
                )
                nc.tensor.matmul(
                    ps[:, :], lhsT=csb["lhsZx"][:, :], rhs=X_sb[:, sl],
                    start=False, stop=False,
                )
                nc.tensor.matmul(
                    ps[:, :], lhsT=csb["lhsZy"][:, :], rhs=Y_sb[:, sl],
                    start=False, stop=False,
                )
                nc.tensor.matmul(
                    ps[:, :], lhsT=csb["lhsZr"][:, :], rhs=r49_sb[0:64, sl],
                    start=False, stop=True,
                )
                nc.scalar.activation(
                    out=zT_sb[:, sl], in_=ps[:, :],
                    func=AF.Identity, bias=csb["zb"][:, :], scale=1.0,
                )

            # ---- logits for all 1024 rows into one PSUM bank [128, 8*10] ----
            pslg = pslgpool.tile([128, NLG * NC], f32, tag="pslg")
            for ch in range(NLG):
                nc.tensor.matmul(
                    pslg[:, ch * NC:(ch + 1) * NC],
                    lhsT=zT_sb[:, ch * 128:(ch + 1) * 128],
                    rhs=csb["w2T"][:, :],
                    start=True, stop=True,
                )
            lg = work.tile([128, NLG, NC], f32, tag="lg")
            # logits + fc2 bias (b2bc broadcast across the chunk dim)
            b2b = csb["b2bc"][:, :]
            b2_bcast = bass.AP(
                tensor=b2b.tensor, offset=b2b.offset,
                ap=[b2b.ap[0], [0, NLG], b2b.ap[1]],
            )
            nc.vector.tensor_tensor(
                out=lg[:, :, :],
                in0=pslg[:, :].rearrange("p (c n) -> p c n", c=NLG),
                in1=b2_bcast, op=Alu.add,
            )
            # log_softmax without max-subtraction (|logits| < 20)
            ex = work.tile([128, NLG, NC], f32, tag="ex")
            nc.scalar.activation(out=ex[:, :, :], in_=lg[:, :, :], func=AF.Exp)
            sm = work.tile([128, NLG], f32, tag="sm")
            nc.vector.tensor_reduce(
                out=sm[:, :], in_=ex[:, :, :], axis=X, op=Alu.add
            )
            lnv = work.tile([128, NLG], f32, tag="lnv")
            nc.scalar.activation(out=lnv[:, :], in_=sm[:, :], func=AF.Ln)
            ot = work.tile([128, NLG, NC], f32, tag="ot")
            lnv_ap = lnv[:, :]
            lnv_bcast = bass.AP(
                tensor=lnv_ap.tensor, offset=lnv_ap.offset,
                ap=[lnv_ap.ap[0], lnv_ap.ap[1], [0, NC]],
            )
            nc.vector.tensor_tensor(
                out=ot[:, :, :], in0=lg[:, :, :], in1=lnv_bcast, op=Alu.subtract
            )
            nc.sync.dma_start(
                out=out_d[:, :],
                in_=ot[:, :, :].rearrange("p c n -> p (c n)"),
            )

    nc.compile()
    _BUILT["nc"] = nc
    return nc


def make_in_maps(x, consts):
    """Shard x over cores; constants replicated."""
    x = np.asarray(x, np.float32)
    in_maps = []
    for c in range(NCORES):
        shard = x[c * BL:(c + 1) * BL]                 # [BL, 512]
        xs = shard.T                                   # [512, BL]
        # layout [128, (h, chunk, HALF)]: each half-batch contiguous
        xTc = np.concatenate(
            [xs[k * 128:(k + 1) * 128, h * HALF:(h + 1) * HALF]
             for h in range(NCHUNK) for k in range(4)],
            axis=1,
        )
        m = {"xT": np.ascontiguousarray(xTc, np.float16)}
        m.update(consts)
        in_maps.append(m)
    return in_maps


def _ensure_axon_hooks():
    """`run_bass_kernel_spmd(trace=True)` under axon imports
    antenv.axon_hooks, which this image lacks. Register a working hook if
    the boot helper is available, else a stub so tracing degrades instead
    of crashing."""
    import sys
    import types

    try:
        import antenv.axon_hooks  # noqa: F401
        return
    except ImportError:
        pass

    hook = None
    try:
        from trn_agent_boot.trn_boot import _ntff_profile_via_ctypes
        import os
        so = "/opt/axon/libaxon_pjrt.so"
        if os.path.exists(so):
            hook = _ntff_profile_via_ctypes(so)
    except Exception:
        hook = None

    m = types.ModuleType("antenv.axon_hooks")
    m.get_axon_ntff_profile_hook = lambda: hook
    m.set_axon_ntff_profile_hook = lambda h: None
    sys.modules["antenv.axon_hooks"] = m


def gather_out(results):
    """Device output is [128, chunk, class]; restore [B, NC] row order."""
    shards = []
    for c in range(NCORES):
        o = np.asarray(results[c]["out"]).reshape(128, NLG, NC)
        shards.append(np.transpose(o, (1, 0, 2)).reshape(BL, NC))
    return np.concatenate(shards, axis=0)


def kernel(x, fc1_w, fc1_b, fc2_w, fc2_b, G, h, A, b):
    from concourse.bass_utils import run_bass_kernel_spmd

    _ensure_axon_hooks()
    consts = _host_precompute(fc1_w, fc1_b, fc2_w, fc2_b, G, h, A, b)
    nc = build_nc()
    in_maps = make_in_maps(x, consts)
    res = run_bass_kernel_spmd(nc, in_maps, core_ids=list(range(NCORES)))
    return gather_out(res.results).astype(np.float32)
